# revision 15
# baseline (speedup 1.0000x reference)
"""Causal multi-head self-attention on 8 TRN2 NeuronCores.

Sharding: tensor-parallel over heads. 16 heads / 8 cores = 2 heads per core.
Each core computes q/k/v projections for its 2 heads (feature-major via
fp32r matmuls), block-causal attention (scores kept k-major so softmax sums
come from a fused ones-column in the attn@v matmul and no transposes are
needed), and a partial output projection against its 128-column slice of
W_O. The host sums the 8 partial outputs.

Layouts on core c (heads 2c, 2c+1 = "A", "B"):
  qT/kT  [128, 2048]  feature-major; rows 0:64 head A dk, 64:128 head B
  vtok   [128, 16, 130] token-major v (PE-transposed) + ones columns at
         64 (A) and 129 (B) so attn@v also produces the softmax denominator
  scoresT[128 k-tok, <=512 q-tok] per (q-tile, k-tile); exp'd on ScalarE
  ctx'   [65, 512] PSUM per head; row 64 = softmax sums
  out    partial [1024, 8192] feature-major; host sums over cores + transposes

Toolchain constraints honored here: col-offset tile_position is illegal for
4-byte matmul dtypes (head B shifted to partitions 64:128 via SBUF->SBUF
DMA before the output projection); fp32r consumers need fp32r-typed
producers; x is transposed on the host so every DMA has a contiguous
innermost run (>=2KB).

The attention k-tile loop is software-pipelined (ctx lags scores by LAG
tiles) so the PE never waits on ScalarE's exp.
"""

import numpy as np
from contextlib import ExitStack

import concourse.bass as bass
import concourse.tile as tile
from concourse import bacc, mybir
from concourse.bass_utils import run_bass_kernel_spmd

F32 = mybir.dt.float32
F32R = mybir.dt.float32r

B, S, D, H = 4, 2048, 1024, 16
DK = D // H  # 64
NCORES = 8
T = B * S  # 8192 tokens
KT = D // 128  # 8 contraction tiles for projections
QTILE = 512  # q-tile width (tokens)
KTILE = 128  # k-tile width (tokens)
NQT = S // QTILE  # 4 q-tiles per batch
NKT = S // KTILE  # 16 k-tiles per batch
LAG = 2  # ctx matmuls trail scores by this many k-tiles
EXP_FUNC = mybir.ActivationFunctionType.Exp
INV_SQRT_DK = 1.0 / np.sqrt(DK)


def build_nc():
    nc = bacc.Bacc("TRN2", target_bir_lowering=False, debug=False)

    xT = nc.dram_tensor("xT", [D, T], F32, kind="ExternalInput").ap()
    wq = nc.dram_tensor("wq", [D, 128], F32, kind="ExternalInput").ap()
    wk = nc.dram_tensor("wk", [D, 128], F32, kind="ExternalInput").ap()
    wv = nc.dram_tensor("wv", [D, 128], F32, kind="ExternalInput").ap()
    wo = nc.dram_tensor("wo", [128, D], F32, kind="ExternalInput").ap()
    tri = nc.dram_tensor("tri", [128, 128], F32, kind="ExternalInput").ap()
    ind = nc.dram_tensor("ind", [2, 128], F32, kind="ExternalInput").ap()
    zk = nc.dram_tensor("zk", [64, S], F32, kind="ExternalInput").ap()
    ident = nc.dram_tensor("ident", [128, 128], F32, kind="ExternalInput").ap()
    outT = nc.dram_tensor("outT", [D, T], F32, kind="ExternalOutput").ap()

    with ExitStack() as ctx:
        tc = ctx.enter_context(tile.TileContext(nc))
        consts = ctx.enter_context(tc.tile_pool(name="consts", bufs=1))
        xt_pool = ctx.enter_context(tc.tile_pool(name="xt_pool", bufs=2))
        batch_pool = ctx.enter_context(tc.tile_pool(name="batch_pool", bufs=2))
        vtmp_pool = ctx.enter_context(tc.tile_pool(name="vtmp_pool", bufs=3))
        exp_pool = ctx.enter_context(tc.tile_pool(name="exp_pool", bufs=6))
        ctxn_pool = ctx.enter_context(tc.tile_pool(name="ctxn_pool", bufs=2))
        tmpb_pool = ctx.enter_context(tc.tile_pool(name="tmpb_pool", bufs=2))
        oall_pool = ctx.enter_context(tc.tile_pool(name="oall_pool", bufs=2))
        small_pool = ctx.enter_context(tc.tile_pool(name="small_pool", bufs=2))
        ps = ctx.enter_context(tc.tile_pool(name="ps", bufs=1, space="PSUM"))

        # --- constants / weights (persistent) ---
        wq_sb = consts.tile([128, KT, 128], F32R)
        nc.sync.dma_start(
            out=wq_sb, in_=wq.rearrange("(kt p) m -> p kt m", p=128).bitcast(F32R)
        )
        wk_sb = consts.tile([128, KT, 128], F32R)
        nc.sync.dma_start(
            out=wk_sb, in_=wk.rearrange("(kt p) m -> p kt m", p=128).bitcast(F32R)
        )
        wv_sb = consts.tile([128, KT, 128], F32R)
        nc.sync.dma_start(
            out=wv_sb, in_=wv.rearrange("(kt p) m -> p kt m", p=128).bitcast(F32R)
        )
        wo_sb = consts.tile([128, KT, 128], F32R)
        nc.sync.dma_start(
            out=wo_sb, in_=wo.rearrange("p (jt m) -> p jt m", jt=KT).bitcast(F32R)
        )
        tri_sb = consts.tile([128, 128], F32R)
        nc.sync.dma_start(out=tri_sb, in_=tri.bitcast(F32R))
        ind_sb = consts.tile([2, 128], F32R)
        nc.sync.dma_start(out=ind_sb, in_=ind.bitcast(F32R))
        ident_sb = consts.tile([128, 128], F32)
        nc.sync.dma_start(out=ident_sb, in_=ident)
        ones_f = consts.tile([128, 64], F32)
        nc.vector.memset(ones_f, 1.0)
        ones_t = consts.tile([128, 64], F32R)
        nc.vector.tensor_copy(ones_t, ones_f)

        pending = []
        for b in range(B):
            tb = b * S  # global token base of this batch

            # --- stage A: q/k/v projections (feature-major) + v transpose ---
            qT_sb = batch_pool.tile([128, S], F32R, name="qT_sb")
            # kT2 half 0: [kA; 0], half 1: [0; kB] — full-K scores keep the
            # whole PE array active so HAM stays at full clock.
            kT2_sb = batch_pool.tile([128, 2, S], F32R, name="kT2_sb")
            nc.sync.dma_start(out=kT2_sb[64:128, 0, :], in_=zk.bitcast(F32R))
            nc.sync.dma_start(out=kT2_sb[0:64, 1, :], in_=zk.bitcast(F32R))
            # vtok: head A at cols 0:64 (+ones at 64), head B at 128:192
            # (+ones at 192); M=128 lhsT slices include junk columns whose
            # PSUM rows are simply never read.
            vtok_sb = batch_pool.tile([128, NKT, 256], F32R, name="vtok_sb")

            for m in range(NKT):
                nc.vector.tensor_copy(vtok_sb[:, m, 64:65], ones_t[:, 0:1])
                nc.vector.tensor_copy(vtok_sb[:, m, 192:193], ones_t[:, 0:1])

            for tt in range(NQT):
                t0 = tb + tt * QTILE
                xt = xt_pool.tile([128, KT, QTILE], F32R, name="xt", tag="xt")
                nc.sync.dma_start(
                    out=xt,
                    in_=xT.rearrange("(kt p) t -> p kt t", p=128)[
                        :, :, t0 : t0 + QTILE
                    ].bitcast(F32R),
                )

                qP = ps.tile([128, QTILE], F32, name="qP", tag="mm", bufs=3)
                for kt in range(KT):
                    nc.tensor.matmul(
                        qP, wq_sb[:, kt, :], xt[:, kt, :], start=(kt == 0), stop=(kt == KT - 1)
                    )
                nc.vector.tensor_copy(qT_sb[:, tt * QTILE : (tt + 1) * QTILE], qP)

                kP = ps.tile([128, QTILE], F32, name="kP", tag="mm", bufs=3)
                for kt in range(KT):
                    nc.tensor.matmul(
                        kP, wk_sb[:, kt, :], xt[:, kt, :], start=(kt == 0), stop=(kt == KT - 1)
                    )
                nc.vector.tensor_copy(
                    kT2_sb[0:64, 0, tt * QTILE : (tt + 1) * QTILE], kP[0:64, :]
                )
                nc.vector.tensor_copy(
                    kT2_sb[64:128, 1, tt * QTILE : (tt + 1) * QTILE], kP[64:128, :]
                )

                vP = ps.tile([128, QTILE], F32, name="vP", tag="mm", bufs=3)
                for kt in range(KT):
                    nc.tensor.matmul(
                        vP, wv_sb[:, kt, :], xt[:, kt, :], start=(kt == 0), stop=(kt == KT - 1)
                    )
                vT_tmp = vtmp_pool.tile([128, QTILE], F32, name="vT_tmp")
                nc.vector.tensor_copy(vT_tmp, vP)
                if pending:
                    pending.pop(0)()
                for s in range(QTILE // 128):
                    vtokP = ps.tile([128, 128], F32, name="vtokP", tag="mm", bufs=3)
                    nc.tensor.transpose(
                        vtokP, vT_tmp[:, s * 128 : (s + 1) * 128], ident_sb
                    )
                    m = tt * 4 + s
                    nc.vector.tensor_copy(vtok_sb[:, m, 0:64], vtokP[:, 0:64])
                    nc.vector.tensor_copy(vtok_sb[:, m, 128:192], vtokP[:, 64:128])

            # --- stage B: attention per q-tile (SW-pipelined over k-tiles;
            # normalization + output projection of tile n deferred into
            # tile n+1's matmul stream so the PE never idles on reciprocal) ---
            for qi in range(NQT):
                q0 = qi * QTILE  # batch-local q base
                nk = 4 * qi + 4  # k-tiles for this q-tile (block-causal)
                ctxA = ps.tile([128, QTILE], F32, name="ctxA", tag="ctxA", bufs=2)
                ctxB = ps.tile([128, QTILE], F32, name="ctxB", tag="ctxB", bufs=2)

                def geom(m, qi=qi):
                    d_off = m - 4 * qi
                    if d_off >= 0:
                        return QTILE - 128 * d_off, 128 * d_off, True
                    return QTILE, 0, False

                exps = {}
                for i in range(nk + LAG):
                    if pending and (i == 1 or i >= 4):
                        pending.pop(0)()
                    if i < nk:
                        m = i
                        width, qoff, diag = geom(m)
                        sP_A = ps.tile([128, QTILE], F32, name="sP_A", tag="mm", bufs=3)
                        sP_B = ps.tile([128, QTILE], F32, name="sP_B", tag="mm", bufs=3)
                        nc.tensor.matmul(
                            sP_A[:, 0:width],
                            kT2_sb[:, 0, m * 128 : (m + 1) * 128],
                            qT_sb[:, q0 + qoff : q0 + QTILE],
                            start=True,
                            stop=True,
                        )
                        nc.tensor.matmul(
                            sP_B[:, 0:width],
                            kT2_sb[:, 1, m * 128 : (m + 1) * 128],
                            qT_sb[:, q0 + qoff : q0 + QTILE],
                            start=True,
                            stop=True,
                        )
                        eA = exp_pool.tile([128, QTILE], F32R, name="eA", tag="exp")
                        eB = exp_pool.tile([128, QTILE], F32R, name="eB", tag="exp")
                        nc.scalar.activation(
                            eA[:, 0:width], sP_A[:, 0:width], EXP_FUNC, scale=INV_SQRT_DK
                        )
                        nc.scalar.activation(
                            eB[:, 0:width], sP_B[:, 0:width], EXP_FUNC, scale=INV_SQRT_DK
                        )
                        if diag:
                            nc.vector.tensor_mul(eA[:, 0:128], eA[:, 0:128], tri_sb)
                            nc.vector.tensor_mul(eB[:, 0:128], eB[:, 0:128], tri_sb)
                        exps[m] = (eA, eB)

                    j = i - LAG
                    if j >= 0:
                        width, qoff, _ = geom(j)
                        first = j == 0
                        last = j == nk - 1
                        eA, eB = exps.pop(j)
                        nc.tensor.matmul(
                            ctxA[:, qoff:QTILE],
                            vtok_sb[:, j, 0:128],
                            eA[:, 0:width],
                            start=first,
                            stop=last,
                            skip_group_check=True,
                        )
                        nc.tensor.matmul(
                            ctxB[:, qoff:QTILE],
                            vtok_sb[:, j, 128:256],
                            eB[:, 0:width],
                            start=first,
                            stop=last,
                            skip_group_check=True,
                        )

                # normalization part 1 (immediate, frees the ctx PSUM slots):
                # copy the two sums rows to SBUF, shift them to partitions
                # 0-1 (tiny SBUF->SBUF DMAs), copy ctx out of PSUM, and shift
                # head B to partitions 64:128.
                s2 = small_pool.tile([65, 2, QTILE], F32R, name="s2")
                nc.scalar.copy(s2[64:65, 0, :], ctxA[64:65, :])
                nc.scalar.copy(s2[64:65, 1, :], ctxB[64:65, :])
                nc.sync.dma_start(out=s2[0:1, 0, :], in_=s2[64:65, 0, :])
                nc.sync.dma_start(out=s2[1:2, 0, :], in_=s2[64:65, 1, :])
                ctxn = ctxn_pool.tile([128, QTILE], F32R, name="ctxn")
                nc.scalar.copy(ctxn[0:64, :], ctxA[0:64, :])
                tmpB = tmpb_pool.tile([64, QTILE], F32R, name="tmpB")
                nc.scalar.copy(tmpB, ctxB[0:64, :])
                nc.sync.dma_start(out=ctxn[64:128, :], in_=tmpB)

                def finish_rb(qi=qi, ctxn=ctxn, s2=s2):
                    # deferred stage 1: K=2 indicator matmul broadcasts both
                    # sums to [128, QTILE] PSUM; reciprocal + normalize.
                    rbP = ps.tile([128, QTILE], F32, name="rbP", tag="mm", bufs=3)
                    nc.tensor.matmul(
                        rbP, ind_sb, s2[0:2, 0, :], start=True, stop=True
                    )
                    rb_sb = small_pool.tile([128, QTILE], F32R, name="rb_sb")
                    with nc.allow_low_precision(reason="softmax denom rounding"):
                        nc.vector.reciprocal(rb_sb, rbP)
                    nc.vector.tensor_mul(ctxn, ctxn, rb_sb)

                def finish_oproj(qi=qi, q0=q0, tb=tb, ctxn=ctxn):
                    # deferred stage 2: output projection (ctxn is normalized
                    # by the time this runs several m-iterations later).
                    o_all = oall_pool.tile([128, KT, QTILE], F32, name="o_all")
                    for jt in range(KT):
                        oP = ps.tile([128, QTILE], F32, name="oP", tag="op", bufs=1)
                        nc.tensor.matmul(
                            oP, wo_sb[:, jt, :], ctxn, start=True, stop=True
                        )
                        nc.any.tensor_copy(o_all[:, jt, :], oP)
                    nc.sync.dma_start(
                        out=outT.rearrange("(jt p) t -> p jt t", p=128)[
                            :, :, tb + q0 : tb + q0 + QTILE
                        ],
                        in_=o_all,
                    )

                pending.extend([finish_rb, finish_oproj])

        while pending:
            pending.pop(0)()

    nc.compile()
    return nc


_NC = None


def _get_nc():
    global _NC
    if _NC is None:
        _NC = build_nc()
    return _NC


def make_in_maps(x, W_Q, W_K, W_V, W_O):
    xTh = np.ascontiguousarray(np.asarray(x, dtype=np.float32).reshape(T, D).T)
    W_Q = np.asarray(W_Q, dtype=np.float32)
    W_K = np.asarray(W_K, dtype=np.float32)
    W_V = np.asarray(W_V, dtype=np.float32)
    W_O = np.asarray(W_O, dtype=np.float32)
    tri = np.triu(np.ones((128, 128), dtype=np.float32))  # tri[k,q]=1 iff q>=k
    zk0 = np.zeros((64, S), dtype=np.float32)
    ind2 = np.zeros((2, 128), dtype=np.float32)
    ind2[0, 0:64] = 1.0
    ind2[1, 64:128] = 1.0
    ident = np.eye(128, dtype=np.float32)
    in_maps = []
    for c in range(NCORES):
        sl = slice(c * 128, (c + 1) * 128)
        in_maps.append(
            {
                "xT": xTh,
                "wq": np.ascontiguousarray(W_Q[sl, :].T),
                "wk": np.ascontiguousarray(W_K[sl, :].T),
                "wv": np.ascontiguousarray(W_V[sl, :].T),
                "wo": np.ascontiguousarray(W_O.T[sl, :]),
                "tri": tri,
                "ind": ind2,
                "zk": zk0,
                "ident": ident,
            }
        )
    return in_maps


def kernel(x, W_Q, W_K, W_V, W_O, _results_hook=None):
    nc = _get_nc()
    in_maps = make_in_maps(x, W_Q, W_K, W_V, W_O)
    res = run_bass_kernel_spmd(nc, in_maps, list(range(NCORES)))
    if _results_hook is not None:
        _results_hook(res)
    acc = np.zeros((D, T), dtype=np.float64)
    for c in range(NCORES):
        acc += res.results[c]["outT"]
    out = np.ascontiguousarray(acc.T).reshape(B, S, D).astype(np.float32)
    return out
